# revision 35
# baseline (speedup 1.0000x reference)
"""Trainium2 Bass kernel for a 7-head dense transformer block.

Strategy: data-parallel over batch (8 batch elements -> 8 NeuronCores, no
collectives). Per core everything runs in a "transposed" activation layout
(features on SBUF partitions, tokens on the free axis), so every matmul's
contraction dim lands on partitions with zero activation transposes.

Single head loop: per head, q/k/v projections, v transpose to natural
layout, then two 1024-query score/exp/PV passes. Scores for two 512-token
chunks land in one 2-bank fp32 PSUM tile so a single ACT exp op covers
1024 queries. Softmax denominators come from bf16 elementwise accumulation
chains on the DVE plus one ones-matmul per 512 queries (instead of PE
ones-matmul accumulation), with the first exp written straight into the
accumulator. The denominator ones-matmuls go to the score-tile PSUM pool
(not the chain pool) so they never block the next head's projection
chains. Reciprocals use the fast custom-DVE approximation; all matmul I/O
is bf16 (same PE rate as f32r, half the bytes), accumulation fp32.
"""

import sys

sys.path.insert(0, "/opt/trn_rl_repo")

import ml_dtypes
import numpy as np

import concourse.bass as bass
import concourse.tile as tile
from concourse import bacc, mybir
from concourse.bass_utils import run_bass_kernel_spmd
from concourse.masks import make_identity

P = 128
DIM = 896            # model dim
HEADS = 7
HD = 128             # head dim
NTOK = 2048          # tokens per batch element
BATCH = 8
CK = DIM // P        # 7 feature chunks
F1 = 2 * DIM         # 1792 ffn hidden
FK = F1 // P         # 14
NJ = NTOK // P       # 16 key-token chunks
NC4 = NTOK // 512    # 4 token chunks
SCALE = HD ** -0.5
EPS = 1e-6

f32 = mybir.dt.float32
bf16 = mybir.dt.bfloat16
AF = mybir.ActivationFunctionType
ALU = mybir.AluOpType


def _build():
    from contextlib import ExitStack

    nc = bacc.Bacc(None, target_bir_lowering=False, debug=False)

    xT = nc.declare_dram_parameter("xT", [DIM, NTOK], bf16, isOutput=False)
    wqkvT = nc.declare_dram_parameter("wqkvT", [DIM, 3 * DIM], bf16, isOutput=False)
    w1T = nc.declare_dram_parameter("w1T", [DIM, F1], bf16, isOutput=False)
    w2T = nc.declare_dram_parameter("w2T", [F1, DIM], bf16, isOutput=False)
    b1p = nc.declare_dram_parameter("b1", [P, FK], f32, isOutput=False)
    b2p = nc.declare_dram_parameter("b2", [P, CK], f32, isOutput=False)
    g1p = nc.declare_dram_parameter("g1", [P, CK], f32, isOutput=False)
    h1p = nc.declare_dram_parameter("h1", [P, CK], f32, isOutput=False)
    g2p = nc.declare_dram_parameter("g2", [P, CK], f32, isOutput=False)
    h2p = nc.declare_dram_parameter("h2", [P, CK], f32, isOutput=False)
    outT = nc.declare_dram_parameter("outT", [DIM, NTOK], bf16, isOutput=True)

    x_r = xT[:].rearrange("(ko p) m -> p ko m", p=P)
    wqkv_r = wqkvT[:].rearrange("(ko p) m -> p ko m", p=P)
    w1_r = w1T[:].rearrange("(ko p) m -> p ko m", p=P)
    w2_r = w2T[:].rearrange("(ko p) m -> p ko m", p=P)
    out_r = outT[:].rearrange("(ko p) m -> p ko m", p=P)

    with tile.TileContext(nc) as tc:
        with ExitStack() as stack:
            pool = lambda **kw: stack.enter_context(tc.tile_pool(**kw))
            cp = pool(name="const", bufs=1)
            resid = pool(name="resid", bufs=4)
            attno = pool(name="attno", bufs=4)
            wqp = pool(name="wq", bufs=6)
            qkp = pool(name="qk", bufs=2)
            vnp = pool(name="vn", bufs=2)
            exp_pool = pool(name="ex", bufs=6)
            accp = pool(name="accp", bufs=4)
            rec_pool = pool(name="rec1", bufs=2)
            h1pool = pool(name="h1pool", bufs=3)
            s2pool = pool(name="s2pool", bufs=3)
            wf = pool(name="wf", bufs=2)
            lnp = pool(name="ln", bufs=1)
            pmm = pool(name="pmm", bufs=2, space="PSUM")
            pst = pool(name="pst", bufs=2, space="PSUM")
            pacc = pool(name="pacc", bufs=2, space="PSUM")

            ident_b = cp.tile([P, P], bf16)
            make_identity(nc, ident_b)
            ones_f = cp.tile([P, P], f32)
            nc.vector.memset(ones_f, 1.0)
            ones_b = cp.tile([P, P], bf16)
            nc.vector.tensor_copy(ones_b, ones_f)
            epss = cp.tile([P, 1], f32)
            nc.vector.memset(epss, EPS)

            def load_xs(n, split=False):
                t = resid.tile([P, CK, 512], bf16, tag="xs", name=f"xs{n}")
                if split:
                    for k in range(CK):
                        nc.sync.dma_start(
                            t[:, k, :], x_r[:, k, n * 512:(n + 1) * 512])
                else:
                    nc.sync.dma_start(t, x_r[:, :, n * 512:(n + 1) * 512])
                return t

            def load_head_w(h):
                tiles = []
                for s in range(3):
                    wt = wqp.tile([P, CK, P], bf16, tag="wq", name=f"wq{h}_{s}")
                    col = s * DIM + h * HD
                    nc.sync.dma_start(wt, wqkv_r[:, :, col:col + HD])
                    tiles.append(wt)
                return tiles

            xs = []
            # attention accumulator (attn out, later z = attn + x), bf16
            xa = [attno.tile([P, CK, 512], bf16, tag="xan", name=f"xa{n}")
                  for n in range(NC4)]

            # ---------------- attention ----------------
            # attention-critical loads first; the small bias/gain tables
            # (only needed from LN1 onward) queue behind them
            head_w = load_head_w(0)
            xs.append(load_xs(0, split=True))
            xs += [load_xs(n) for n in range(1, NC4)]
            b1s = cp.tile([P, FK], f32)
            nc.sync.dma_start(b1s, b1p[:])
            b2s = cp.tile([P, CK], f32)
            nc.sync.dma_start(b2s, b2p[:])
            g1s = cp.tile([P, CK], f32)
            nc.sync.dma_start(g1s, g1p[:])
            h1s = cp.tile([P, CK], f32)
            nc.sync.dma_start(h1s, h1p[:])
            g2s = cp.tile([P, CK], f32)
            nc.sync.dma_start(g2s, g2p[:])
            h2s = cp.tile([P, CK], f32)
            nc.sync.dma_start(h2s, h2p[:])
            for h in range(HEADS):
                cur_w = head_w
                if h + 1 < HEADS:
                    head_w = load_head_w(h + 1)
                # n-outer so the first head's chains track the xs DMA
                # arrival order chunk by chunk
                qkv = [qkp.tile([P, NTOK], bf16, tag=f"qkv{s}",
                                name=f"qkv{h}_{s}") for s in range(3)]
                for n in range(NC4):
                    for s in range(3):
                        ps = pmm.tile([P, 512], f32, tag="mm")
                        for k in range(CK):
                            nc.tensor.matmul(
                                ps,
                                lhsT=cur_w[s][:, k, :],
                                rhs=xs[n][:, k, :],
                                start=(k == 0),
                                stop=(k == CK - 1),
                            )
                        nc.vector.tensor_copy(
                            qkv[s][:, n * 512:(n + 1) * 512], ps)
                qT, kT, vT = qkv

                v_nat = vnp.tile([P, NJ, P], bf16, tag="vnat")
                for jc in range(NJ):
                    pstt = pmm.tile([P, 512], bf16, tag="mm",
                                    name="pstt")[:, :P]
                    nc.tensor.transpose(
                        pstt, vT[:, jc * P:(jc + 1) * P], ident_b)
                    nc.vector.tensor_copy(v_nat[:, jc, :], pstt)

                for icp in range(2):
                    q0 = icp * 1024
                    xap = [pacc.tile([P, 512], f32, tag="xa", name="xap")
                           for _ in range(2)]
                    acc = accp.tile([P, 1024], bf16, tag="acc")
                    for jc in range(NJ):
                        st = pst.tile([P, 1024], f32, tag="st")
                        for i2 in range(2):
                            nc.tensor.matmul(
                                st[:, i2 * 512:(i2 + 1) * 512],
                                lhsT=kT[:, jc * P:(jc + 1) * P],
                                rhs=qT[:, q0 + i2 * 512:q0 + (i2 + 1) * 512],
                                start=True,
                                stop=True,
                            )
                        if jc == 0:
                            # first exp writes the accumulator directly
                            ex = acc
                            nc.scalar.activation(ex, st, AF.Exp, scale=SCALE)
                        else:
                            ex = exp_pool.tile([P, 1024], bf16, tag="ex")
                            nc.scalar.activation(ex, st, AF.Exp, scale=SCALE)
                            nc.vector.tensor_add(acc, acc, ex)
                        for i2 in range(2):
                            nc.tensor.matmul(
                                xap[i2],
                                lhsT=v_nat[:, jc, :],
                                rhs=ex[:, i2 * 512:(i2 + 1) * 512],
                                start=(jc == 0), stop=(jc == NJ - 1))
                    rec = rec_pool.tile([P, 1024], f32, tag="rec")
                    # both denominator matmuls share ONE score-pool tile
                    # (bank-aligned halves), so the other slot stays free
                    # for the next pass's first score matmul.
                    rst = pst.tile([P, 1024], f32, tag="st", name="rs")
                    for i2 in range(2):
                        rs = rst[:, i2 * 512:(i2 + 1) * 512]
                        nc.tensor.matmul(
                            rs, lhsT=ones_b,
                            rhs=acc[:, i2 * 512:(i2 + 1) * 512],
                            start=True, stop=True)
                        nc.vector.reciprocal_approx_fast(
                            rec[:, i2 * 512:(i2 + 1) * 512], rs)
                        nc.vector.tensor_mul(
                            xa[icp * 2 + i2][:, h, :], xap[i2],
                            rec[:, i2 * 512:(i2 + 1) * 512])

            # ---------------- LayerNorm helper ----------------
            def layer_norm(src_c, dst_c, g, b):
                """src_c/dst_c: lists of [P, CK, W] bf16 chunk views, W <= 512."""
                for n in range(len(src_c)):
                    W = src_c[n].shape[2]
                    smsq = pst.tile([P, 1024], f32, tag="st", name="smsq")
                    sm = smsq[:, 0:W]
                    sq = smsq[:, 512:512 + W]
                    for k in range(CK):
                        nc.tensor.matmul(sm, lhsT=ones_b,
                                         rhs=src_c[n][:, k, :],
                                         start=(k == 0), stop=(k == CK - 1))
                    for k in range(CK):
                        xsq = lnp.tile([P, 512], bf16, tag="xsq", name="xsq")[:, :W]
                        nc.scalar.activation(xsq, src_c[n][:, k, :], AF.Square)
                        nc.tensor.matmul(sq, lhsT=ones_b, rhs=xsq,
                                         start=(k == 0), stop=(k == CK - 1))
                    mu = lnp.tile([P, 512], f32, tag="mu", name="mu")[:, :W]
                    nc.vector.tensor_scalar_mul(mu, sm, 1.0 / DIM)
                    mu2 = lnp.tile([P, 512], f32, tag="mu2", name="mu2")[:, :W]
                    nc.vector.tensor_mul(mu2, mu, mu)
                    var = lnp.tile([P, 512], f32, tag="var", name="var")[:, :W]
                    nc.vector.scalar_tensor_tensor(
                        out=var, in0=sq, scalar=1.0 / DIM, in1=mu2,
                        op0=ALU.mult, op1=ALU.subtract)
                    sd = mu2
                    nc.scalar.activation(sd, var, AF.Sqrt, bias=epss)
                    rstd = var
                    nc.vector.reciprocal_approx_fast(rstd, sd)
                    A = lnp.tile([P, 512], bf16, tag="A", name="A")[:, :W]
                    nc.vector.tensor_copy(A, rstd)
                    B = lnp.tile([P, 512], bf16, tag="B", name="B")[:, :W]
                    nc.vector.scalar_tensor_tensor(
                        out=B, in0=mu, scalar=-1.0, in1=rstd,
                        op0=ALU.mult, op1=ALU.mult)
                    for k in range(CK):
                        t = lnp.tile([P, 512], bf16, tag="t", name="tt")[:, :W]
                        nc.vector.tensor_mul(t, src_c[n][:, k, :], A)
                        nc.vector.tensor_add(t, t, B)
                        nc.vector.tensor_scalar(
                            out=dst_c[n][:, k, :], in0=t,
                            scalar1=g[:, k:k + 1], scalar2=b[:, k:k + 1],
                            op0=ALU.mult, op1=ALU.add,
                        )

            # ---------------- LN1 (z in xa; hT into fresh xs slots) -------
            # chunks 0,1 first; chunks 2,3 are emitted later so their DVE
            # chains hide under the superchunk-0 FFN1 matmuls
            hT = [None] * NC4
            for n in range(NC4):
                nc.vector.tensor_add(xa[n][:], xa[n][:], xs[n][:])
            for n in range(NC4):
                hT[n] = resid.tile([P, CK, 512], bf16, tag="xs",
                                   name=f"hT{n}")
            layer_norm(xa[0:2], hT[0:2], g1s, h1s)

            # ---------------- FFN + LN2 ----------------
            def ffn1_m(sc, h1c, m, chunks=(0, 1)):
                wt = wf.tile([P, FK, P], bf16, tag="wf",
                             name="w1t")[:, :CK, :]
                nc.sync.dma_start(wt, w1_r[:, :, m * P:(m + 1) * P])
                for n5 in chunks:
                    nch = sc * 2 + n5
                    ps = pmm.tile([P, 512], f32, tag="mm", name="f1ps")
                    for k in range(CK):
                        nc.tensor.matmul(
                            ps, lhsT=wt[:, k, :], rhs=hT[nch][:, k, :],
                            start=(k == 0), stop=(k == CK - 1))
                    nc.scalar.activation(
                        h1c[n5][:, m, :], ps, AF.Gelu, bias=b1s[:, m:m + 1])

            def ffn2_mo(sc, n5, h1c, s2c, mo):
                wt = wf.tile([P, FK, P], bf16, tag="wf", name="w2t")
                nc.sync.dma_start(wt, w2_r[:, :, mo * P:(mo + 1) * P])
                nch = sc * 2 + n5
                # pacc is idle once attention is done: giving FFN2 its own
                # pool keeps FFN1 (pmm) and FFN2 chains from contending
                ps = pacc.tile([P, 512], f32, tag="xa", name="f2ps")
                for k in range(FK):
                    nc.tensor.matmul(
                        ps, lhsT=wt[:, k, :], rhs=h1c[n5][:, k, :],
                        start=(k == 0), stop=(k == FK - 1))
                nc.vector.scalar_tensor_tensor(
                    out=s2c[n5][:, mo, :], in0=ps,
                    scalar=b2s[:, mo:mo + 1],
                    in1=hT[nch][:, mo, :],
                    op0=ALU.add, op1=ALU.add,
                )

            def ln2_store(s2c, nch, n5, last=False):
                if last:
                    halves = [s2c[n5][:, :, 0:256], s2c[n5][:, :, 256:512]]
                    layer_norm(halves, halves, g2s, h2s)
                    for hv in range(2):
                        nc.sync.dma_start(
                            out_r[:, :, nch * 512 + hv * 256:
                                  nch * 512 + (hv + 1) * 256],
                            halves[hv])
                else:
                    layer_norm(s2c[n5:n5 + 1], s2c[n5:n5 + 1], g2s, h2s)
                    nc.sync.dma_start(
                        out_r[:, :, nch * 512:(nch + 1) * 512], s2c[n5])

            for sc in range(2):  # two 1024-token super-chunks
                h1c = [h1pool.tile([P, FK, 512], bf16, tag="h1",
                                   name=f"h1_{sc}_{i}") for i in range(2)]
                s2c = [s2pool.tile([P, CK, 512], bf16, tag="s2",
                                   name=f"s2_{sc}_{i}") for i in range(2)]
                if sc == 0:
                    for m in range(FK):
                        ffn1_m(sc, h1c, m)
                        if m == 1:
                            layer_norm(xa[2:4], hT[2:4], g1s, h1s)
                else:
                    # chunk-major in the tail so chunk 2's FFN2/LN2/store
                    # overlaps chunk 3's FFN1/FFN2
                    for m in range(FK):
                        ffn1_m(sc, h1c, m, chunks=(0,))
                    for m in range(FK):
                        ffn1_m(sc, h1c, m, chunks=(1,))
                for mo in range(CK):
                    ffn2_mo(sc, 0, h1c, s2c, mo)
                ln2_store(s2c, sc * 2, 0)
                for mo in range(CK):
                    ffn2_mo(sc, 1, h1c, s2c, mo)
                ln2_store(s2c, sc * 2 + 1, 1, last=(sc == 1))

    nc.compile()
    return nc


_NC = None


def make_in_maps(inputs):
    x = np.asarray(inputs["x"], np.float32)
    qkv_w = np.asarray(inputs["qkv_w"], np.float32)
    proj1_w = np.asarray(inputs["proj1_w"], np.float32)
    proj1_b = np.asarray(inputs["proj1_b"], np.float32)
    proj2_w = np.asarray(inputs["proj2_w"], np.float32)
    proj2_b = np.asarray(inputs["proj2_b"], np.float32)
    ln1_g = np.asarray(inputs["ln1_g"], np.float32)
    ln1_b = np.asarray(inputs["ln1_b"], np.float32)
    ln2_g = np.asarray(inputs["ln2_g"], np.float32)
    ln2_b = np.asarray(inputs["ln2_b"], np.float32)

    bf = ml_dtypes.bfloat16
    common = {
        "wqkvT": np.ascontiguousarray(qkv_w.T).astype(bf),
        "w1T": np.ascontiguousarray(proj1_w.T).astype(bf),
        "w2T": np.ascontiguousarray(proj2_w.T).astype(bf),
        "b1": np.ascontiguousarray(proj1_b.reshape(FK, P).T),
        "b2": np.ascontiguousarray(proj2_b.reshape(CK, P).T),
        "g1": np.ascontiguousarray(ln1_g.reshape(CK, P).T),
        "h1": np.ascontiguousarray(ln1_b.reshape(CK, P).T),
        "g2": np.ascontiguousarray(ln2_g.reshape(CK, P).T),
        "h2": np.ascontiguousarray(ln2_b.reshape(CK, P).T),
    }
    return [
        dict(common, xT=np.ascontiguousarray(x[b].T).astype(bf))
        for b in range(BATCH)
    ]


def kernel(**inputs):
    global _NC
    if _NC is None:
        _NC = _build()
    nc = _NC

    in_maps = make_in_maps(inputs)
    res = run_bass_kernel_spmd(nc, in_maps, core_ids=list(range(BATCH)))
    out = np.stack(
        [res.results[b]["outT"].astype(np.float32).T for b in range(BATCH)],
        axis=0)
    return np.ascontiguousarray(out, dtype=np.float32)


if __name__ == "__main__":
    rng = np.random.default_rng(0)
    demo = {
        "x": rng.standard_normal((BATCH, NTOK, DIM), dtype=np.float32),
        "qkv_w": rng.standard_normal((3 * DIM, DIM), dtype=np.float32) * 0.03,
        "proj1_w": rng.standard_normal((F1, DIM), dtype=np.float32) * 0.03,
        "proj1_b": rng.standard_normal((F1,), dtype=np.float32) * 0.03,
        "proj2_w": rng.standard_normal((DIM, F1), dtype=np.float32) * 0.03,
        "proj2_b": rng.standard_normal((DIM,), dtype=np.float32) * 0.03,
        "ln1_g": np.ones(DIM, np.float32),
        "ln1_b": np.zeros(DIM, np.float32),
        "ln2_g": np.ones(DIM, np.float32),
        "ln2_b": np.zeros(DIM, np.float32),
    }
    y = kernel(**demo)
    print(y.shape, y.dtype)


# revision 36
# speedup vs baseline: 1.0124x; 1.0124x over previous
"""Trainium2 Bass kernel for a 7-head dense transformer block.

Strategy: data-parallel over batch (8 batch elements -> 8 NeuronCores, no
collectives). Per core everything runs in a "transposed" activation layout
(features on SBUF partitions, tokens on the free axis), so every matmul's
contraction dim lands on partitions with zero activation transposes.

Single head loop: per head, q/k/v projections, v transpose to natural
layout, then two 1024-query score/exp/PV passes. Scores for two 512-token
chunks land in one 2-bank fp32 PSUM tile so a single ACT exp op covers
1024 queries. Softmax denominators come from bf16 elementwise accumulation
chains on the DVE plus one ones-matmul per 512 queries (instead of PE
ones-matmul accumulation), with the first exp written straight into the
accumulator. The denominator ones-matmuls go to the score-tile PSUM pool
(not the chain pool) so they never block the next head's projection
chains. Reciprocals use the fast custom-DVE approximation; all matmul I/O
is bf16 (same PE rate as f32r, half the bytes), accumulation fp32.
"""

import sys

sys.path.insert(0, "/opt/trn_rl_repo")

import ml_dtypes
import numpy as np

import concourse.bass as bass
import concourse.tile as tile
from concourse import bacc, mybir
from concourse.bass_utils import run_bass_kernel_spmd
from concourse.masks import make_identity

P = 128
DIM = 896            # model dim
HEADS = 7
HD = 128             # head dim
NTOK = 2048          # tokens per batch element
BATCH = 8
CK = DIM // P        # 7 feature chunks
F1 = 2 * DIM         # 1792 ffn hidden
FK = F1 // P         # 14
NJ = NTOK // P       # 16 key-token chunks
NC4 = NTOK // 512    # 4 token chunks
SCALE = HD ** -0.5
EPS = 1e-6

f32 = mybir.dt.float32
bf16 = mybir.dt.bfloat16
AF = mybir.ActivationFunctionType
ALU = mybir.AluOpType


def _build():
    from contextlib import ExitStack

    nc = bacc.Bacc(None, target_bir_lowering=False, debug=False)

    xT = nc.declare_dram_parameter("xT", [DIM, NTOK], bf16, isOutput=False)
    wqkvT = nc.declare_dram_parameter("wqkvT", [DIM, 3 * DIM], bf16, isOutput=False)
    w1T = nc.declare_dram_parameter("w1T", [DIM, F1], bf16, isOutput=False)
    w2T = nc.declare_dram_parameter("w2T", [F1, DIM], bf16, isOutput=False)
    b1p = nc.declare_dram_parameter("b1", [P, FK], f32, isOutput=False)
    b2p = nc.declare_dram_parameter("b2", [P, CK], f32, isOutput=False)
    g1p = nc.declare_dram_parameter("g1", [P, CK], f32, isOutput=False)
    h1p = nc.declare_dram_parameter("h1", [P, CK], f32, isOutput=False)
    g2p = nc.declare_dram_parameter("g2", [P, CK], f32, isOutput=False)
    h2p = nc.declare_dram_parameter("h2", [P, CK], f32, isOutput=False)
    outT = nc.declare_dram_parameter("outT", [DIM, NTOK], bf16, isOutput=True)

    x_r = xT[:].rearrange("(ko p) m -> p ko m", p=P)
    wqkv_r = wqkvT[:].rearrange("(ko p) m -> p ko m", p=P)
    w1_r = w1T[:].rearrange("(ko p) m -> p ko m", p=P)
    w2_r = w2T[:].rearrange("(ko p) m -> p ko m", p=P)
    out_r = outT[:].rearrange("(ko p) m -> p ko m", p=P)

    with tile.TileContext(nc) as tc:
        with ExitStack() as stack:
            pool = lambda **kw: stack.enter_context(tc.tile_pool(**kw))
            cp = pool(name="const", bufs=1)
            resid = pool(name="resid", bufs=4)
            attno = pool(name="attno", bufs=4)
            wqp = pool(name="wq", bufs=6)
            qkp = pool(name="qk", bufs=2)
            vnp = pool(name="vn", bufs=2)
            exp_pool = pool(name="ex", bufs=6)
            accp = pool(name="accp", bufs=4)
            rec_pool = pool(name="rec1", bufs=2)
            h1pool = pool(name="h1pool", bufs=3)
            s2pool = pool(name="s2pool", bufs=3)
            wf = pool(name="wf", bufs=2)
            lnp = pool(name="ln", bufs=1)
            pmm = pool(name="pmm", bufs=2, space="PSUM")
            pst = pool(name="pst", bufs=2, space="PSUM")
            pacc = pool(name="pacc", bufs=2, space="PSUM")

            ident_b = cp.tile([P, P], bf16)
            make_identity(nc, ident_b)
            ones_f = cp.tile([P, P], f32)
            nc.vector.memset(ones_f, 1.0)
            ones_b = cp.tile([P, P], bf16)
            nc.vector.tensor_copy(ones_b, ones_f)
            epss = cp.tile([P, 1], f32)
            nc.vector.memset(epss, EPS)
            b1s = cp.tile([P, FK], f32)
            nc.sync.dma_start(b1s, b1p[:])
            b2s = cp.tile([P, CK], f32)
            nc.sync.dma_start(b2s, b2p[:])
            g1s = cp.tile([P, CK], f32)
            nc.sync.dma_start(g1s, g1p[:])
            h1s = cp.tile([P, CK], f32)
            nc.sync.dma_start(h1s, h1p[:])
            g2s = cp.tile([P, CK], f32)
            nc.sync.dma_start(g2s, g2p[:])
            h2s = cp.tile([P, CK], f32)
            nc.sync.dma_start(h2s, h2p[:])

            def load_xs(n, split=False):
                t = resid.tile([P, CK, 512], bf16, tag="xs", name=f"xs{n}")
                if split:
                    for k in range(CK):
                        nc.sync.dma_start(
                            t[:, k, :], x_r[:, k, n * 512:(n + 1) * 512])
                else:
                    nc.sync.dma_start(t, x_r[:, :, n * 512:(n + 1) * 512])
                return t

            def load_head_w(h):
                tiles = []
                for s in range(3):
                    wt = wqp.tile([P, CK, P], bf16, tag="wq", name=f"wq{h}_{s}")
                    col = s * DIM + h * HD
                    nc.sync.dma_start(wt, wqkv_r[:, :, col:col + HD])
                    tiles.append(wt)
                return tiles

            xs = []
            # attention accumulator (attn out, later z = attn + x), bf16
            xa = [attno.tile([P, CK, 512], bf16, tag="xan", name=f"xa{n}")
                  for n in range(NC4)]

            # ---------------- attention ----------------
            head_w = load_head_w(0)
            xs.append(load_xs(0, split=True))
            xs += [load_xs(n) for n in range(1, NC4)]
            for h in range(HEADS):
                cur_w = head_w
                if h + 1 < HEADS:
                    head_w = load_head_w(h + 1)
                # n-outer so the first head's chains track the xs DMA
                # arrival order chunk by chunk
                qkv = [qkp.tile([P, NTOK], bf16, tag=f"qkv{s}",
                                name=f"qkv{h}_{s}") for s in range(3)]
                for n in range(NC4):
                    for s in range(3):
                        ps = pmm.tile([P, 512], f32, tag="mm")
                        for k in range(CK):
                            nc.tensor.matmul(
                                ps,
                                lhsT=cur_w[s][:, k, :],
                                rhs=xs[n][:, k, :],
                                start=(k == 0),
                                stop=(k == CK - 1),
                            )
                        nc.vector.tensor_copy(
                            qkv[s][:, n * 512:(n + 1) * 512], ps)
                qT, kT, vT = qkv

                v_nat = vnp.tile([P, NJ, P], bf16, tag="vnat")
                for jc in range(NJ):
                    pstt = pmm.tile([P, 512], bf16, tag="mm",
                                    name="pstt")[:, :P]
                    nc.tensor.transpose(
                        pstt, vT[:, jc * P:(jc + 1) * P], ident_b)
                    nc.vector.tensor_copy(v_nat[:, jc, :], pstt)

                for icp in range(2):
                    q0 = icp * 1024
                    xap = [pacc.tile([P, 512], f32, tag="xa", name="xap")
                           for _ in range(2)]
                    acc = accp.tile([P, 1024], bf16, tag="acc")
                    for jc in range(NJ):
                        st = pst.tile([P, 1024], f32, tag="st")
                        for i2 in range(2):
                            nc.tensor.matmul(
                                st[:, i2 * 512:(i2 + 1) * 512],
                                lhsT=kT[:, jc * P:(jc + 1) * P],
                                rhs=qT[:, q0 + i2 * 512:q0 + (i2 + 1) * 512],
                                start=True,
                                stop=True,
                            )
                        if jc == 0:
                            # first exp writes the accumulator directly
                            ex = acc
                            nc.scalar.activation(ex, st, AF.Exp, scale=SCALE)
                        else:
                            ex = exp_pool.tile([P, 1024], bf16, tag="ex")
                            nc.scalar.activation(ex, st, AF.Exp, scale=SCALE)
                            nc.vector.tensor_add(acc, acc, ex)
                        for i2 in range(2):
                            nc.tensor.matmul(
                                xap[i2],
                                lhsT=v_nat[:, jc, :],
                                rhs=ex[:, i2 * 512:(i2 + 1) * 512],
                                start=(jc == 0), stop=(jc == NJ - 1))
                    rec = rec_pool.tile([P, 1024], f32, tag="rec")
                    for i2 in range(2):
                        # rs lives in the score-tile pool: its slots cycle
                        # fast, so holding one through the reciprocal never
                        # blocks the next head's projection chains.
                        rs = pst.tile([P, 1024], f32, tag="st",
                                      name="rs")[:, :512]
                        nc.tensor.matmul(
                            rs, lhsT=ones_b,
                            rhs=acc[:, i2 * 512:(i2 + 1) * 512],
                            start=True, stop=True)
                        nc.vector.reciprocal_approx_fast(
                            rec[:, i2 * 512:(i2 + 1) * 512], rs)
                        nc.vector.tensor_mul(
                            xa[icp * 2 + i2][:, h, :], xap[i2],
                            rec[:, i2 * 512:(i2 + 1) * 512])

            # ---------------- LayerNorm helper ----------------
            def layer_norm(src_c, dst_c, g, b):
                """src_c/dst_c: lists of [P, CK, W] bf16 chunk views, W <= 512."""
                for n in range(len(src_c)):
                    W = src_c[n].shape[2]
                    smsq = pst.tile([P, 1024], f32, tag="st", name="smsq")
                    sm = smsq[:, 0:W]
                    sq = smsq[:, 512:512 + W]
                    for k in range(CK):
                        nc.tensor.matmul(sm, lhsT=ones_b,
                                         rhs=src_c[n][:, k, :],
                                         start=(k == 0), stop=(k == CK - 1))
                    for k in range(CK):
                        xsq = lnp.tile([P, 512], bf16, tag="xsq", name="xsq")[:, :W]
                        nc.scalar.activation(xsq, src_c[n][:, k, :], AF.Square)
                        nc.tensor.matmul(sq, lhsT=ones_b, rhs=xsq,
                                         start=(k == 0), stop=(k == CK - 1))
                    mu = lnp.tile([P, 512], f32, tag="mu", name="mu")[:, :W]
                    nc.vector.tensor_scalar_mul(mu, sm, 1.0 / DIM)
                    mu2 = lnp.tile([P, 512], f32, tag="mu2", name="mu2")[:, :W]
                    nc.vector.tensor_mul(mu2, mu, mu)
                    var = lnp.tile([P, 512], f32, tag="var", name="var")[:, :W]
                    nc.vector.scalar_tensor_tensor(
                        out=var, in0=sq, scalar=1.0 / DIM, in1=mu2,
                        op0=ALU.mult, op1=ALU.subtract)
                    sd = mu2
                    nc.scalar.activation(sd, var, AF.Sqrt, bias=epss)
                    rstd = var
                    nc.vector.reciprocal_approx_fast(rstd, sd)
                    A = lnp.tile([P, 512], bf16, tag="A", name="A")[:, :W]
                    nc.vector.tensor_copy(A, rstd)
                    B = lnp.tile([P, 512], bf16, tag="B", name="B")[:, :W]
                    nc.vector.scalar_tensor_tensor(
                        out=B, in0=mu, scalar=-1.0, in1=rstd,
                        op0=ALU.mult, op1=ALU.mult)
                    for k in range(CK):
                        t = lnp.tile([P, 512], bf16, tag="t", name="tt")[:, :W]
                        nc.vector.tensor_mul(t, src_c[n][:, k, :], A)
                        nc.vector.tensor_add(t, t, B)
                        nc.vector.tensor_scalar(
                            out=dst_c[n][:, k, :], in0=t,
                            scalar1=g[:, k:k + 1], scalar2=b[:, k:k + 1],
                            op0=ALU.mult, op1=ALU.add,
                        )

            # ---------------- LN1 (z in xa; hT into fresh xs slots) -------
            # chunks 0,1 first; chunks 2,3 are emitted later so their DVE
            # chains hide under the superchunk-0 FFN1 matmuls
            hT = [None] * NC4
            for n in range(NC4):
                nc.vector.tensor_add(xa[n][:], xa[n][:], xs[n][:])
            for n in range(NC4):
                hT[n] = resid.tile([P, CK, 512], bf16, tag="xs",
                                   name=f"hT{n}")
            layer_norm(xa[0:2], hT[0:2], g1s, h1s)

            # ---------------- FFN + LN2 ----------------
            def ffn1_m(sc, h1c, m, chunks=(0, 1)):
                wt = wf.tile([P, FK, P], bf16, tag="wf",
                             name="w1t")[:, :CK, :]
                nc.sync.dma_start(wt, w1_r[:, :, m * P:(m + 1) * P])
                for n5 in chunks:
                    nch = sc * 2 + n5
                    ps = pmm.tile([P, 512], f32, tag="mm", name="f1ps")
                    for k in range(CK):
                        nc.tensor.matmul(
                            ps, lhsT=wt[:, k, :], rhs=hT[nch][:, k, :],
                            start=(k == 0), stop=(k == CK - 1))
                    nc.scalar.activation(
                        h1c[n5][:, m, :], ps, AF.Gelu, bias=b1s[:, m:m + 1])

            def ffn2_mo(sc, n5, h1c, s2c, mo):
                wt = wf.tile([P, FK, P], bf16, tag="wf", name="w2t")
                nc.sync.dma_start(wt, w2_r[:, :, mo * P:(mo + 1) * P])
                nch = sc * 2 + n5
                ps = pmm.tile([P, 512], f32, tag="mm", name="f2ps")
                for k in range(FK):
                    nc.tensor.matmul(
                        ps, lhsT=wt[:, k, :], rhs=h1c[n5][:, k, :],
                        start=(k == 0), stop=(k == FK - 1))
                nc.vector.scalar_tensor_tensor(
                    out=s2c[n5][:, mo, :], in0=ps,
                    scalar=b2s[:, mo:mo + 1],
                    in1=hT[nch][:, mo, :],
                    op0=ALU.add, op1=ALU.add,
                )

            def ln2_store(s2c, nch, n5, last=False):
                if last:
                    halves = [s2c[n5][:, :, 0:256], s2c[n5][:, :, 256:512]]
                    layer_norm(halves, halves, g2s, h2s)
                    for hv in range(2):
                        nc.sync.dma_start(
                            out_r[:, :, nch * 512 + hv * 256:
                                  nch * 512 + (hv + 1) * 256],
                            halves[hv])
                else:
                    layer_norm(s2c[n5:n5 + 1], s2c[n5:n5 + 1], g2s, h2s)
                    nc.sync.dma_start(
                        out_r[:, :, nch * 512:(nch + 1) * 512], s2c[n5])

            for sc in range(2):  # two 1024-token super-chunks
                h1c = [h1pool.tile([P, FK, 512], bf16, tag="h1",
                                   name=f"h1_{sc}_{i}") for i in range(2)]
                s2c = [s2pool.tile([P, CK, 512], bf16, tag="s2",
                                   name=f"s2_{sc}_{i}") for i in range(2)]
                if sc == 0:
                    for m in range(FK):
                        ffn1_m(sc, h1c, m)
                        if m == 1:
                            layer_norm(xa[2:4], hT[2:4], g1s, h1s)
                else:
                    # chunk-major in the tail so chunk 2's FFN2/LN2/store
                    # overlaps chunk 3's FFN1/FFN2
                    for m in range(FK):
                        ffn1_m(sc, h1c, m, chunks=(0,))
                    for m in range(FK):
                        ffn1_m(sc, h1c, m, chunks=(1,))
                for mo in range(CK):
                    ffn2_mo(sc, 0, h1c, s2c, mo)
                ln2_store(s2c, sc * 2, 0)
                for mo in range(CK):
                    ffn2_mo(sc, 1, h1c, s2c, mo)
                ln2_store(s2c, sc * 2 + 1, 1, last=(sc == 1))

    nc.compile()
    return nc


_NC = None


def make_in_maps(inputs):
    x = np.asarray(inputs["x"], np.float32)
    qkv_w = np.asarray(inputs["qkv_w"], np.float32)
    proj1_w = np.asarray(inputs["proj1_w"], np.float32)
    proj1_b = np.asarray(inputs["proj1_b"], np.float32)
    proj2_w = np.asarray(inputs["proj2_w"], np.float32)
    proj2_b = np.asarray(inputs["proj2_b"], np.float32)
    ln1_g = np.asarray(inputs["ln1_g"], np.float32)
    ln1_b = np.asarray(inputs["ln1_b"], np.float32)
    ln2_g = np.asarray(inputs["ln2_g"], np.float32)
    ln2_b = np.asarray(inputs["ln2_b"], np.float32)

    bf = ml_dtypes.bfloat16
    common = {
        "wqkvT": np.ascontiguousarray(qkv_w.T).astype(bf),
        "w1T": np.ascontiguousarray(proj1_w.T).astype(bf),
        "w2T": np.ascontiguousarray(proj2_w.T).astype(bf),
        "b1": np.ascontiguousarray(proj1_b.reshape(FK, P).T),
        "b2": np.ascontiguousarray(proj2_b.reshape(CK, P).T),
        "g1": np.ascontiguousarray(ln1_g.reshape(CK, P).T),
        "h1": np.ascontiguousarray(ln1_b.reshape(CK, P).T),
        "g2": np.ascontiguousarray(ln2_g.reshape(CK, P).T),
        "h2": np.ascontiguousarray(ln2_b.reshape(CK, P).T),
    }
    return [
        dict(common, xT=np.ascontiguousarray(x[b].T).astype(bf))
        for b in range(BATCH)
    ]


def kernel(**inputs):
    global _NC
    if _NC is None:
        _NC = _build()
    nc = _NC

    in_maps = make_in_maps(inputs)
    res = run_bass_kernel_spmd(nc, in_maps, core_ids=list(range(BATCH)))
    out = np.stack(
        [res.results[b]["outT"].astype(np.float32).T for b in range(BATCH)],
        axis=0)
    return np.ascontiguousarray(out, dtype=np.float32)


if __name__ == "__main__":
    rng = np.random.default_rng(0)
    demo = {
        "x": rng.standard_normal((BATCH, NTOK, DIM), dtype=np.float32),
        "qkv_w": rng.standard_normal((3 * DIM, DIM), dtype=np.float32) * 0.03,
        "proj1_w": rng.standard_normal((F1, DIM), dtype=np.float32) * 0.03,
        "proj1_b": rng.standard_normal((F1,), dtype=np.float32) * 0.03,
        "proj2_w": rng.standard_normal((DIM, F1), dtype=np.float32) * 0.03,
        "proj2_b": rng.standard_normal((DIM,), dtype=np.float32) * 0.03,
        "ln1_g": np.ones(DIM, np.float32),
        "ln1_b": np.zeros(DIM, np.float32),
        "ln2_g": np.ones(DIM, np.float32),
        "ln2_b": np.zeros(DIM, np.float32),
    }
    y = kernel(**demo)
    print(y.shape, y.dtype)
